# revision 17
# baseline (speedup 1.0000x reference)
"""Multi-head attention (B=2, S=2048, D=1024, H=16, dh=64) on 8 TRN2 NeuronCores.

Sharding: data-parallel over batch (2) x tensor-parallel over heads (4 per core).
Core c handles batch c//4 and heads [4*(c%4), 4*(c%4)+4). Each core computes a
partial output (its heads' contribution through Wo); the host sums the 4 partials
per batch and adds bo (the unshard step for a sum-sharded tensor).

Matmuls run in bf16 with f32 PSUM accumulation: bf16 keeps the PE at full rate
with standalone (reorder-hidden) weight loads, where fp32/fp32r pay a serialized
2-cycle-per-column LDWEIGHTS on every matmul and starve the HAM clock gate.

The PV matmul carries an extra ones-column in the stationary operand so the
softmax denominator falls out of the same accumulation for free; bv is pre-added
to V (P @ (V + 1*bv) = PV + denom*bv, so the post-divide result already includes
bv). Normalization is a single DVE divide against the partition-broadcast
denominator. Every intermediate is produced in the layout its consumer wants
(host supplies transposed activations; attn^T lands exactly as the Wo stationary),
so there are no on-device transposes.
"""

import sys

if "/opt/trn_rl_repo" not in sys.path:
    sys.path.insert(0, "/opt/trn_rl_repo")

import ml_dtypes
import numpy as np

import concourse.bass as bass
import concourse.mybir as mybir
import concourse.tile as tile
from concourse import bacc, bass_utils
from concourse.bass import ts

# Problem constants (hardcoded per contract)
B, S, D = 2, 2048, 1024
H, DH = 16, 64            # total heads, head dim
HC = 4                    # heads per core
DHC = HC * DH             # 256 projected dims per core
NCORES = 8
P = 128
CH = 512                  # query-chunk for attention / projection sub-chunk
NCH = S // CH             # 4
TT = S // P               # 16 key tiles
KO = D // P               # 8 contraction tiles for projections

f32 = mybir.dt.float32
bf16 = mybir.dt.bfloat16
EXP = mybir.ActivationFunctionType.Exp
DIV = mybir.AluOpType.divide

_compiled = None          # cached nc across calls
last_results = None       # BassKernelResults of the most recent run (for profiling)


def _build():
    nc = bacc.Bacc("TRN2", target_bir_lowering=False, debug=False)

    # Per-core DRAM parameters. Activations are pre-transposed on host.
    qT = nc.dram_tensor("qT", [D, S], bf16, kind="ExternalInput")
    kT = nc.dram_tensor("kT", [D, S], bf16, kind="ExternalInput")
    vT = nc.dram_tensor("vT", [D, S], bf16, kind="ExternalInput")
    wq = nc.dram_tensor("wq", [D, DHC], bf16, kind="ExternalInput")
    wk = nc.dram_tensor("wk", [D, DHC], bf16, kind="ExternalInput")
    wv = nc.dram_tensor("wv", [D, DHC], bf16, kind="ExternalInput")
    wo = nc.dram_tensor("wo", [DHC, D], bf16, kind="ExternalInput")
    bq = nc.dram_tensor("bq", [DHC], f32, kind="ExternalInput")
    bk = nc.dram_tensor("bk", [DHC], f32, kind="ExternalInput")
    bv = nc.dram_tensor("bv", [DHC], f32, kind="ExternalInput")
    out = nc.dram_tensor("out", [S, D], f32, kind="ExternalOutput")

    qT_r = qT.ap().rearrange("(ko p) s -> p ko s", p=P)
    kT_r = kT.ap().rearrange("(ko p) s -> p ko s", p=P)
    vT_r = vT.ap().rearrange("(ko p) s -> p ko s", p=P)

    with tile.TileContext(nc) as tc:
        with (
            tc.tile_pool(name="weights", bufs=1) as wpool,
            tc.tile_pool(name="acts", bufs=1) as apool,
            tc.tile_pool(name="xin", bufs=3) as xpool,
            tc.tile_pool(name="pt", bufs=5) as ptpool,
            tc.tile_pool(name="small", bufs=2) as spool,
            tc.tile_pool(name="outs", bufs=3) as opool,
            tc.tile_pool(name="proj_ps", bufs=2, space="PSUM") as proj_ps,
            tc.tile_pool(name="l_ps", bufs=2, space="PSUM") as l_ps,
            tc.tile_pool(name="o_ps", bufs=2, space="PSUM") as o_ps,
        ):
            # ---- constants / weights ----
            wq_sb = wpool.tile([P, KO, DHC], bf16, tag="wq")
            wk_sb = wpool.tile([P, KO, DHC], bf16, tag="wk")
            wv_sb = wpool.tile([P, KO, DHC], bf16, tag="wv")
            wo_sb = wpool.tile([P, DHC // P, D], bf16, tag="wo")
            nc.sync.dma_start(out=wq_sb, in_=wq.ap().rearrange("(ko p) m -> p ko m", p=P))
            nc.sync.dma_start(out=wk_sb, in_=wk.ap().rearrange("(ko p) m -> p ko m", p=P))
            nc.sync.dma_start(out=wv_sb, in_=wv.ap().rearrange("(ko p) m -> p ko m", p=P))
            nc.sync.dma_start(out=wo_sb, in_=wo.ap().rearrange("(ko p) n -> p ko n", p=P))
            bq_sb = wpool.tile([P, 2], f32, tag="bq")
            bk_sb = wpool.tile([P, 2], f32, tag="bk")
            nc.sync.dma_start(out=bq_sb, in_=bq.ap().rearrange("(mo p) -> p mo", p=P))
            nc.sync.dma_start(out=bk_sb, in_=bk.ap().rearrange("(mo p) -> p mo", p=P))
            # bv broadcast across partitions: folded into V so the normalize
            # divide lands on already-biased values (P@(V+1*bv) = PV+denom*bv).
            bv_row = wpool.tile([P, DHC], f32, tag="bv_row")
            nc.sync.dma_start(out=bv_row[0:1, :], in_=bv.ap().rearrange("(a d) -> a d", a=1))
            bv_bc = wpool.tile([P, DHC], f32, tag="bv_bc")
            nc.gpsimd.partition_broadcast(bv_bc, bv_row[0:1, :])
            bv_heads = bv_bc[:, :].rearrange("p (h c) -> p h c", c=DH)

            # ---- activation tiles ----
            # q^T/k^T: [P, m, S] where projected dim r lives at (r % 128, r // 128)
            q_sb = apool.tile([P, 2, S], bf16, tag="q")
            k_sb = apool.tile([P, 2, S], bf16, tag="k")
            # v natural + ones column per head: [P, tt, 4*65 (+63 pad so every
            # head's stationary slice can be 128 columns wide -> fast weight load)]
            VW = HC * (DH + 1)
            v_sb = apool.tile([P, TT, VW + P - (DH + 1)], bf16, tag="v")
            # attn^T accumulator, same layout as q_sb
            attn_sb = apool.tile([P, 2, S], bf16, tag="attn")
            v_heads = v_sb[:, :, 0:VW].rearrange("p tt (h c) -> p tt h c", c=DH + 1)
            ones_f32 = wpool.tile([P, TT, HC], f32, tag="ones")
            nc.vector.memset(ones_f32, 1.0)
            nc.vector.tensor_copy(out=v_heads[:, :, :, DH], in_=ones_f32)
            nc.vector.memset(v_sb[:, :, VW:], 0.0)

            # ---- phase 1: projections ----
            # k first (attention needs ALL of k before its first chunk), then
            # q, then v (v is only consumed by PV, which trails exp anyway).
            for c in range(NCH):
                sl = slice(c * CH, (c + 1) * CH)
                xk = xpool.tile([P, KO, CH], bf16, tag="x", name=f"xk{c}")
                nc.sync.dma_start(out=xk, in_=kT_r[:, :, sl])
                for m in range(2):
                    ps = proj_ps.tile([P, CH], f32, tag="proj")
                    for ko in range(KO):
                        nc.tensor.matmul(ps, wk_sb[:, ko, ts(m, P)], xk[:, ko, :],
                                         start=(ko == 0), stop=(ko == KO - 1))
                    nc.vector.tensor_scalar_add(out=k_sb[:, m, sl], in0=ps,
                                                scalar1=bk_sb[:, m : m + 1])
            for c in range(NCH):
                sl = slice(c * CH, (c + 1) * CH)
                xq = xpool.tile([P, KO, CH], bf16, tag="x", name=f"xq{c}")
                nc.sync.dma_start(out=xq, in_=qT_r[:, :, sl])
                for m in range(2):
                    ps = proj_ps.tile([P, CH], f32, tag="proj")
                    for ko in range(KO):
                        nc.tensor.matmul(ps, wq_sb[:, ko, ts(m, P)], xq[:, ko, :],
                                         start=(ko == 0), stop=(ko == KO - 1))
                    nc.vector.tensor_scalar_add(out=q_sb[:, m, sl], in0=ps,
                                                scalar1=bq_sb[:, m : m + 1])
            for c in range(NCH):
                sl = slice(c * CH, (c + 1) * CH)
                # v: natural layout; this chunk covers t rows [c*CH, c*CH+CH)
                xv = xpool.tile([P, KO, CH], bf16, tag="x", name=f"xv{c}")
                nc.sync.dma_start(out=xv, in_=vT_r[:, :, sl])
                for th in range(CH // P):
                    tt = (c * CH) // P + th
                    ps = proj_ps.tile([P, DHC], f32, tag="proj")
                    for ko in range(KO):
                        nc.tensor.matmul(ps, xv[:, ko, ts(th, P)], wv_sb[:, ko, :],
                                         start=(ko == 0), stop=(ko == KO - 1))
                    # scatter 4 heads' 64 cols into the 65-strided layout, +bv
                    nc.vector.tensor_add(
                        out=v_heads[:, tt, :, 0:DH],
                        in0=ps.rearrange("p (h c) -> p h c", c=DH),
                        in1=bv_heads,
                    )

            # ---- phase 2: attention ----
            # Pipelined over head PAIRS. QK matmuls of the pair strictly
            # alternate partition bases 0/64 (adjacent alternating K=64
            # matmuls run concurrently in distinct PE row groups: measured
            # 119ns/mm vs 429 for a same-base stream). The previous pair's
            # PV matmuls are woven between QK units so PE rides along while
            # ACT (exp) paces the phase.
            pairs = [(cidx, m) for cidx in range(NCH) for m in range(2)]
            pts = {}

            def pv_mm(cidx, h, po, tt):
                # Stationary is a full 128-column slice of v_sb (64 V cols +
                # ones + pad) so the weight path runs in fast/background mode;
                # rows 65.. of the PSUM are garbage and never read.
                nc.tensor.matmul(
                    po,
                    v_sb[:, tt, (DH + 1) * h : (DH + 1) * h + P],
                    pts[(cidx, h)][:, tt, :],
                    start=(tt == 0), stop=(tt == TT - 1),
                )

            def pv_finish(cidx, h, po):
                csl = slice(cidx * CH, (cidx + 1) * CH)
                base = DH * (h % 2)
                m = h // 2
                # evacuate PSUM fast (one wide copy), then normalize from
                # SBUF so the bank frees for the next head's PV
                tmp = spool.tile([P, CH], f32, tag="tmp")
                nc.vector.tensor_copy(out=tmp[0 : DH + 1, :], in_=po[0 : DH + 1, :])
                rec = spool.tile([P, CH], f32, tag="rec")
                nc.vector.reciprocal(out=rec[0:1, :], in_=tmp[DH : DH + 1, :])
                bc = spool.tile([P, CH], f32, tag="bc")
                nc.gpsimd.partition_broadcast(bc[0:DH, :], rec[0:1, :])
                nc.vector.tensor_mul(
                    out=attn_sb[base : base + DH, m, csl],
                    in0=tmp[0:DH, :], in1=bc[0:DH, :],
                )

            prev = None
            for cidx, m in pairs:
                csl = slice(cidx * CH, (cidx + 1) * CH)
                h0, h1 = 2 * m, 2 * m + 1
                for h in (h0, h1):
                    pts[(cidx, h)] = ptpool.tile([P, TT, CH], bf16, tag="pt",
                                                 name=f"pt_c{cidx}_h{h}")
                po0 = po1 = None
                if prev is not None:
                    pcidx, pm = prev
                    po0 = o_ps.tile([P, CH], f32, tag="o", name=f"po_{cidx}_{m}a")
                    po1 = o_ps.tile([P, CH], f32, tag="o", name=f"po_{cidx}_{m}b")
                for tb in range(TT // 2):
                    ps0 = l_ps.tile([P, 2, CH], f32, tag="l", name=f"l{cidx}_{m}_{tb}a")
                    ps1 = l_ps.tile([P, 2, CH], f32, tag="l", name=f"l{cidx}_{m}_{tb}b")
                    for j in range(2):
                        tt = 2 * tb + j
                        nc.tensor.matmul(ps0[:, j, :], k_sb[0:DH, m, ts(tt, P)],
                                         q_sb[0:DH, m, csl], start=True, stop=True)
                        nc.tensor.matmul(ps1[:, j, :], k_sb[DH:P, m, ts(tt, P)],
                                         q_sb[DH:P, m, csl], start=True, stop=True)
                    nc.scalar.activation(out=pts[(cidx, h0)][:, 2 * tb : 2 * tb + 2, :],
                                         in_=ps0, func=EXP)
                    nc.scalar.activation(out=pts[(cidx, h1)][:, 2 * tb : 2 * tb + 2, :],
                                         in_=ps1, func=EXP)
                    if prev is not None:
                        for j in range(2):
                            tt = 2 * tb + j
                            pv_mm(pcidx, 2 * pm, po0, tt)
                            pv_mm(pcidx, 2 * pm + 1, po1, tt)
                if prev is not None:
                    pv_finish(pcidx, 2 * pm, po0)
                    pv_finish(pcidx, 2 * pm + 1, po1)
                prev = (cidx, m)
            pcidx, pm = prev
            po0 = o_ps.tile([P, CH], f32, tag="o", name="po_lastA")
            po1 = o_ps.tile([P, CH], f32, tag="o", name="po_lastB")
            for tt in range(TT):
                pv_mm(pcidx, 2 * pm, po0, tt)
                pv_mm(pcidx, 2 * pm + 1, po1, tt)
            pv_finish(pcidx, 2 * pm, po0)
            pv_finish(pcidx, 2 * pm + 1, po1)

            # ---- phase 3: output projection ----
            for st in range(S // P):
                for n in range(2):
                    pw = proj_ps.tile([P, 512], f32, tag="proj")
                    for ko in range(2):
                        nc.tensor.matmul(pw, attn_sb[:, ko, ts(st, P)],
                                         wo_sb[:, ko, ts(n, 512)],
                                         start=(ko == 0), stop=(ko == 1))
                    ot = opool.tile([P, 512], f32, tag="ot")
                    nc.vector.tensor_copy(out=ot, in_=pw)
                    nc.sync.dma_start(out=out.ap()[ts(st, P), ts(n, 512)], in_=ot)

    nc.finalize()
    return nc


def kernel(**inputs):
    global _compiled, last_results
    if _compiled is None:
        _compiled = _build()
    nc = _compiled

    query = np.asarray(inputs["query"], np.float32)
    key = np.asarray(inputs["key"], np.float32)
    value = np.asarray(inputs["value"], np.float32)
    Wq = np.asarray(inputs["Wq"], np.float32)
    Wk = np.asarray(inputs["Wk"], np.float32)
    Wv = np.asarray(inputs["Wv"], np.float32)
    Wo = np.asarray(inputs["Wo"], np.float32)
    bq_f = np.asarray(inputs["bq"], np.float32)
    bk_f = np.asarray(inputs["bk"], np.float32)
    bv_f = np.asarray(inputs["bv"], np.float32)
    bo_f = np.asarray(inputs["bo"], np.float32)

    bf = ml_dtypes.bfloat16
    scale = 1.0 / np.sqrt(np.float32(DH))
    qT = [np.ascontiguousarray(query[b].T).astype(bf) for b in range(B)]
    kT = [np.ascontiguousarray(key[b].T).astype(bf) for b in range(B)]
    vT = [np.ascontiguousarray(value[b].T).astype(bf) for b in range(B)]

    in_maps = []
    for c in range(NCORES):
        b = c // 4
        sh = c % 4
        sl = slice(DHC * sh, DHC * (sh + 1))
        in_maps.append({
            "qT": qT[b], "kT": kT[b], "vT": vT[b],
            "wq": (Wq[:, sl] * scale).astype(bf),
            "wk": np.ascontiguousarray(Wk[:, sl]).astype(bf),
            "wv": np.ascontiguousarray(Wv[:, sl]).astype(bf),
            "wo": np.ascontiguousarray(Wo[sl, :]).astype(bf),
            "bq": np.ascontiguousarray(bq_f[sl]) * scale,
            "bk": np.ascontiguousarray(bk_f[sl]),
            "bv": np.ascontiguousarray(bv_f[sl]),
        })

    res = bass_utils.run_bass_kernel_spmd(nc, in_maps, core_ids=list(range(NCORES)))
    last_results = res

    final = np.empty((B, S, D), np.float32)
    for b in range(B):
        acc = res.results[4 * b]["out"].astype(np.float32)
        for sh in range(1, 4):
            acc = acc + res.results[4 * b + sh]["out"]
        final[b] = acc + bo_f
    return final


# revision 18
# speedup vs baseline: 1.1199x; 1.1199x over previous
"""Multi-head attention (B=2, S=2048, D=1024, H=16, dh=64) on 8 TRN2 NeuronCores.

Sharding: data-parallel over batch (2) x tensor-parallel over heads (4 per core).
Core c handles batch c//4 and heads [4*(c%4), 4*(c%4)+4). Each core computes a
partial output (its heads' contribution through Wo); the host sums the 4 partials
per batch and adds bo (the unshard step for a sum-sharded tensor).

Matmuls run in bf16 with f32 PSUM accumulation: bf16 keeps the PE at full rate
with standalone (reorder-hidden) weight loads, where fp32/fp32r pay a serialized
2-cycle-per-column LDWEIGHTS on every matmul and starve the HAM clock gate.

The PV matmul carries an extra ones-column in the stationary operand so the
softmax denominator falls out of the same accumulation for free; bv is pre-added
to V (P @ (V + 1*bv) = PV + denom*bv, so the post-divide result already includes
bv). Normalization is a single DVE divide against the partition-broadcast
denominator. Every intermediate is produced in the layout its consumer wants
(host supplies transposed activations; attn^T lands exactly as the Wo stationary),
so there are no on-device transposes.
"""

import sys

if "/opt/trn_rl_repo" not in sys.path:
    sys.path.insert(0, "/opt/trn_rl_repo")

import ml_dtypes
import numpy as np

import concourse.bass as bass
import concourse.mybir as mybir
import concourse.tile as tile
from concourse import bacc, bass_utils
from concourse.bass import ts

# Problem constants (hardcoded per contract)
B, S, D = 2, 2048, 1024
H, DH = 16, 64            # total heads, head dim
HC = 4                    # heads per core
DHC = HC * DH             # 256 projected dims per core
NCORES = 8
P = 128
CH = 512                  # query-chunk for attention / projection sub-chunk
NCH = S // CH             # 4
TT = S // P               # 16 key tiles
KO = D // P               # 8 contraction tiles for projections

f32 = mybir.dt.float32
bf16 = mybir.dt.bfloat16
EXP = mybir.ActivationFunctionType.Exp
DIV = mybir.AluOpType.divide

_compiled = None          # cached nc across calls
last_results = None       # BassKernelResults of the most recent run (for profiling)


def _build():
    nc = bacc.Bacc("TRN2", target_bir_lowering=False, debug=False)

    # Per-core DRAM parameters. Activations are pre-transposed on host.
    qT = nc.dram_tensor("qT", [D, S], bf16, kind="ExternalInput")
    kT = nc.dram_tensor("kT", [D, S], bf16, kind="ExternalInput")
    vT = nc.dram_tensor("vT", [D, S], bf16, kind="ExternalInput")
    wq = nc.dram_tensor("wq", [D, DHC], bf16, kind="ExternalInput")
    wk = nc.dram_tensor("wk", [D, DHC], bf16, kind="ExternalInput")
    wv = nc.dram_tensor("wv", [D, DHC], bf16, kind="ExternalInput")
    wo = nc.dram_tensor("wo", [DHC, D], bf16, kind="ExternalInput")
    bq = nc.dram_tensor("bq", [DHC], f32, kind="ExternalInput")
    bk = nc.dram_tensor("bk", [DHC], f32, kind="ExternalInput")
    bv = nc.dram_tensor("bv", [DHC], f32, kind="ExternalInput")
    out = nc.dram_tensor("out", [S, D], f32, kind="ExternalOutput")

    qT_r = qT.ap().rearrange("(ko p) s -> p ko s", p=P)
    kT_r = kT.ap().rearrange("(ko p) s -> p ko s", p=P)
    vT_r = vT.ap().rearrange("(ko p) s -> p ko s", p=P)

    with tile.TileContext(nc) as tc:
        with (
            tc.tile_pool(name="weights", bufs=1) as wpool,
            tc.tile_pool(name="acts", bufs=1) as apool,
            tc.tile_pool(name="xin", bufs=3) as xpool,
            tc.tile_pool(name="pt", bufs=5) as ptpool,
            tc.tile_pool(name="small", bufs=2) as spool,
            tc.tile_pool(name="outs", bufs=3) as opool,
            tc.tile_pool(name="proj_ps", bufs=2, space="PSUM") as proj_ps,
            tc.tile_pool(name="l_ps", bufs=2, space="PSUM") as l_ps,
            tc.tile_pool(name="o_ps", bufs=2, space="PSUM") as o_ps,
        ):
            # ---- constants / weights ----
            wq_sb = wpool.tile([P, KO, DHC], bf16, tag="wq")
            wk_sb = wpool.tile([P, KO, DHC], bf16, tag="wk")
            wv_sb = wpool.tile([P, KO, DHC], bf16, tag="wv")
            wo_sb = wpool.tile([P, DHC // P, D], bf16, tag="wo")
            nc.sync.dma_start(out=wq_sb, in_=wq.ap().rearrange("(ko p) m -> p ko m", p=P))
            nc.sync.dma_start(out=wk_sb, in_=wk.ap().rearrange("(ko p) m -> p ko m", p=P))
            nc.sync.dma_start(out=wv_sb, in_=wv.ap().rearrange("(ko p) m -> p ko m", p=P))
            nc.sync.dma_start(out=wo_sb, in_=wo.ap().rearrange("(ko p) n -> p ko n", p=P))
            bq_sb = wpool.tile([P, 2], f32, tag="bq")
            bk_sb = wpool.tile([P, 2], f32, tag="bk")
            nc.sync.dma_start(out=bq_sb, in_=bq.ap().rearrange("(mo p) -> p mo", p=P))
            nc.sync.dma_start(out=bk_sb, in_=bk.ap().rearrange("(mo p) -> p mo", p=P))
            # bv broadcast across partitions: folded into V so the normalize
            # divide lands on already-biased values (P@(V+1*bv) = PV+denom*bv).
            bv_row = wpool.tile([P, DHC], f32, tag="bv_row")
            nc.sync.dma_start(out=bv_row[0:1, :], in_=bv.ap().rearrange("(a d) -> a d", a=1))
            bv_bc = wpool.tile([P, DHC], f32, tag="bv_bc")
            nc.gpsimd.partition_broadcast(bv_bc, bv_row[0:1, :])
            bv_heads = bv_bc[:, :].rearrange("p (h c) -> p h c", c=DH)

            # ---- activation tiles ----
            # q^T/k^T: [P, m, S] where projected dim r lives at (r % 128, r // 128)
            q_sb = apool.tile([P, 2, S], bf16, tag="q")
            k_sb = apool.tile([P, 2, S], bf16, tag="k")
            # v natural + ones column per head: [P, tt, 4*65 (+63 pad so every
            # head's stationary slice can be 128 columns wide -> fast weight load)]
            VW = HC * (DH + 1)
            v_sb = apool.tile([P, TT, VW + P - (DH + 1)], bf16, tag="v")
            # attn^T accumulator, same layout as q_sb
            attn_sb = apool.tile([P, 2, S], bf16, tag="attn")
            v_heads = v_sb[:, :, 0:VW].rearrange("p tt (h c) -> p tt h c", c=DH + 1)
            ones_f32 = wpool.tile([P, TT, HC], f32, tag="ones")
            nc.vector.memset(ones_f32, 1.0)
            nc.vector.tensor_copy(out=v_heads[:, :, :, DH], in_=ones_f32)
            nc.vector.memset(v_sb[:, :, VW:], 0.0)

            # ---- phase 1: projections ----
            # k first (attention needs ALL of k before its first chunk), then
            # q, then v (v is only consumed by PV, which trails exp anyway).
            for c in range(NCH):
                sl = slice(c * CH, (c + 1) * CH)
                xk = xpool.tile([P, KO, CH], bf16, tag="x", name=f"xk{c}")
                nc.sync.dma_start(out=xk, in_=kT_r[:, :, sl])
                for m in range(2):
                    ps = proj_ps.tile([P, CH], f32, tag="proj")
                    for ko in range(KO):
                        nc.tensor.matmul(ps, wk_sb[:, ko, ts(m, P)], xk[:, ko, :],
                                         start=(ko == 0), stop=(ko == KO - 1))
                    nc.vector.tensor_scalar_add(out=k_sb[:, m, sl], in0=ps,
                                                scalar1=bk_sb[:, m : m + 1])
            for c in range(NCH):
                sl = slice(c * CH, (c + 1) * CH)
                xq = xpool.tile([P, KO, CH], bf16, tag="x", name=f"xq{c}")
                nc.sync.dma_start(out=xq, in_=qT_r[:, :, sl])
                for m in range(2):
                    ps = proj_ps.tile([P, CH], f32, tag="proj")
                    for ko in range(KO):
                        nc.tensor.matmul(ps, wq_sb[:, ko, ts(m, P)], xq[:, ko, :],
                                         start=(ko == 0), stop=(ko == KO - 1))
                    nc.vector.tensor_scalar_add(out=q_sb[:, m, sl], in0=ps,
                                                scalar1=bq_sb[:, m : m + 1])
            for c in range(NCH):
                sl = slice(c * CH, (c + 1) * CH)
                # v: natural layout; this chunk covers t rows [c*CH, c*CH+CH)
                xv = xpool.tile([P, KO, CH], bf16, tag="x", name=f"xv{c}")
                nc.sync.dma_start(out=xv, in_=vT_r[:, :, sl])
                for th in range(CH // P):
                    tt = (c * CH) // P + th
                    ps = proj_ps.tile([P, DHC], f32, tag="proj")
                    for ko in range(KO):
                        nc.tensor.matmul(ps, xv[:, ko, ts(th, P)], wv_sb[:, ko, :],
                                         start=(ko == 0), stop=(ko == KO - 1))
                    # scatter 4 heads' 64 cols into the 65-strided layout, +bv
                    nc.vector.tensor_add(
                        out=v_heads[:, tt, :, 0:DH],
                        in0=ps.rearrange("p (h c) -> p h c", c=DH),
                        in1=bv_heads,
                    )

            # ---- phase 2: attention ----
            # Emit ALL heads' QK+exp before any PV within a chunk: ACT (exp)
            # is the pacer in this phase, and the in-order PE queue must
            # always hold work that doesn't depend on the exp in flight.
            for cidx in range(NCH):
                csl = slice(cidx * CH, (cidx + 1) * CH)
                pts = {}
                for h in range(HC):
                    pts[h] = ptpool.tile([P, TT, CH], bf16, tag="pt",
                                         name=f"pt_c{cidx}_h{h}")
                    base = DH * (h % 2)
                    m = h // 2
                    for tb in range(TT // 2):
                        ps = l_ps.tile([P, 2, CH], f32, tag="l")
                        for j in range(2):
                            tt = 2 * tb + j
                            nc.tensor.matmul(
                                ps[:, j, :],
                                k_sb[base : base + DH, m, ts(tt, P)],
                                q_sb[base : base + DH, m, csl],
                                start=True, stop=True,
                            )
                        nc.scalar.activation(out=pts[h][:, 2 * tb : 2 * tb + 2, :],
                                             in_=ps, func=EXP)
                # PV (+ denominator via the ones column)
                for h in range(HC):
                    base = DH * (h % 2)
                    m = h // 2
                    po = o_ps.tile([P, CH], f32, tag="o")
                    for tt in range(TT):
                        nc.tensor.matmul(
                            po[0 : DH + 1, :],
                            v_heads[:, tt, h, :],
                            pts[h][:, tt, :],
                            start=(tt == 0), stop=(tt == TT - 1),
                        )
                    # evacuate PSUM fast (one wide copy), then normalize
                    # from SBUF so the bank frees for the next head's PV
                    tmp = spool.tile([P, CH], f32, tag="tmp")
                    nc.vector.tensor_copy(out=tmp[0 : DH + 1, :], in_=po[0 : DH + 1, :])
                    rec = spool.tile([P, CH], f32, tag="rec")
                    nc.vector.reciprocal(out=rec[0:1, :], in_=tmp[DH : DH + 1, :])
                    bc = spool.tile([P, CH], f32, tag="bc")
                    nc.gpsimd.partition_broadcast(bc[0:DH, :], rec[0:1, :])
                    nc.vector.tensor_mul(
                        out=attn_sb[base : base + DH, m, csl],
                        in0=tmp[0:DH, :], in1=bc[0:DH, :],
                    )

            # ---- phase 3: output projection ----
            for st in range(S // P):
                for n in range(2):
                    pw = proj_ps.tile([P, 512], f32, tag="proj")
                    for ko in range(2):
                        nc.tensor.matmul(pw, attn_sb[:, ko, ts(st, P)],
                                         wo_sb[:, ko, ts(n, 512)],
                                         start=(ko == 0), stop=(ko == 1))
                    ot = opool.tile([P, 512], f32, tag="ot")
                    nc.vector.tensor_copy(out=ot, in_=pw)
                    nc.sync.dma_start(out=out.ap()[ts(st, P), ts(n, 512)], in_=ot)

    nc.finalize()
    return nc


def kernel(**inputs):
    global _compiled, last_results
    if _compiled is None:
        _compiled = _build()
    nc = _compiled

    query = np.asarray(inputs["query"], np.float32)
    key = np.asarray(inputs["key"], np.float32)
    value = np.asarray(inputs["value"], np.float32)
    Wq = np.asarray(inputs["Wq"], np.float32)
    Wk = np.asarray(inputs["Wk"], np.float32)
    Wv = np.asarray(inputs["Wv"], np.float32)
    Wo = np.asarray(inputs["Wo"], np.float32)
    bq_f = np.asarray(inputs["bq"], np.float32)
    bk_f = np.asarray(inputs["bk"], np.float32)
    bv_f = np.asarray(inputs["bv"], np.float32)
    bo_f = np.asarray(inputs["bo"], np.float32)

    bf = ml_dtypes.bfloat16
    scale = 1.0 / np.sqrt(np.float32(DH))
    qT = [np.ascontiguousarray(query[b].T).astype(bf) for b in range(B)]
    kT = [np.ascontiguousarray(key[b].T).astype(bf) for b in range(B)]
    vT = [np.ascontiguousarray(value[b].T).astype(bf) for b in range(B)]

    in_maps = []
    for c in range(NCORES):
        b = c // 4
        sh = c % 4
        sl = slice(DHC * sh, DHC * (sh + 1))
        in_maps.append({
            "qT": qT[b], "kT": kT[b], "vT": vT[b],
            "wq": (Wq[:, sl] * scale).astype(bf),
            "wk": np.ascontiguousarray(Wk[:, sl]).astype(bf),
            "wv": np.ascontiguousarray(Wv[:, sl]).astype(bf),
            "wo": np.ascontiguousarray(Wo[sl, :]).astype(bf),
            "bq": np.ascontiguousarray(bq_f[sl]) * scale,
            "bk": np.ascontiguousarray(bk_f[sl]),
            "bv": np.ascontiguousarray(bv_f[sl]),
        })

    res = bass_utils.run_bass_kernel_spmd(nc, in_maps, core_ids=list(range(NCORES)))
    last_results = res

    final = np.empty((B, S, D), np.float32)
    for b in range(B):
        acc = res.results[4 * b]["out"].astype(np.float32)
        for sh in range(1, 4):
            acc = acc + res.results[4 * b + sh]["out"]
        final[b] = acc + bo_f
    return final
